# revision 17
# baseline (speedup 1.0000x reference)
"""Trainium2 Bass kernel for uniform cubic B-spline basis (Cox-de Boor, degree 3).

Uniform knots => all 252 basis functions are shifts of one cardinal cubic C(s)
on [0,4). Row r is zero except columns cstart..cstart+3 (cstart = clip(i-3, 0,
248), i = floor(u), u = (x+pi)/h), holding C(u-cstart-m).

Strategy (v2, dense + bitwise masks): the original kernel scatter-wrote
68-float windows with the Q7 `dma_scatter_add` ucode; its descriptor
generation (~7.5ns/token * 131072 tokens ~ 1ms) was the bottleneck. Instead,
each output row is materialized DENSELY in SBUF as a 256-wide bf16 slot
(cols 252..255 padding) and streamed out with plain contiguous HWDGE DMA
(67MB/core, DMA engines measured at ~25B/ns each).

Per row: q = cstart>>4 picks one of 16 16-col blocks; w' = (u-cstart) +
(cstart&15) is the eval point of a 20-wide window win[j] = C(w' - j) whose
4-wide support sits in [of16, of16+3] (spans at most blocks q, q+1). The
cubic: y=|s-2|, C = relu(2-y)^3/6 - (2/3)relu(1-y)^3; the CUBES run fully on
the Act engine as exp(3*ln(z) + ln(1/6)) / exp(3*ln(t) + ln(2/3)) (7 Act ops
per chunk), so DVE does only the s-grid subtract and the final subtract.

Dense assembly uses BITWISE masking on uint32 lanes (2 bf16 cols per u32
element, halving element count vs bf16 masking): block q' of a slot =
win_u32[0:8] AND mbits[q'], where mbits = 0xFFFFFFFF iff q==q' (host-built,
streamed per chunk). The win[16:20] spill is OR'd into the first 2 u32 words
of block q'+1 (those words are 0 from the AND when the spill is nonzero, so
OR is exact; q'==15 never spills because right-edge clipping keeps of16<=8).
Bitwise ops are DVE-only (NCC_EBIR039: unsupported on the Pool engine), so
all masking runs on DVE; the overlapping spill OR sits behind a
self-semaphore so the strided-AP overlap is an explicit edge (Bacc's
dependency tracker does not model strided-AP overlaps and may reorder
instructions whose dependency it cannot see). The f32 window pipeline is
kept on DVE with abs/relu/Square on Act: offloading streaming tensor ops to
GPSIMD was tried and REGRESSED (343us -> 397us) because the Pool engine
shares its SBUF port with DVE; Pool must stay off streaming work. The 16
block writes are fused into one 4D-AP tensor_tensor to save instruction
overhead.

Row r maps partition-major (p = r // F, f = r % F) so each partition's chunk
is one contiguous 32KB DRAM run. Host prep is index arithmetic only
(u/cstart/w'/mask words); host post slices [:, :252] and casts bf16->f32
(values are computed in f32 and rounded once => rel err ~2e-3).
"""
import sys
import types

sys.path.insert(0, "/opt/trn_rl_repo")
sys.path.insert(0, "/root/.axon_site/_ro/trn_rl_repo")

import numpy as np


def _ensure_axon_hooks():
    if "antenv.axon_hooks" in sys.modules:
        return
    try:
        import antenv
    except ImportError:
        return
    m = types.ModuleType("antenv.axon_hooks")
    m._hook = None
    m.set_axon_ntff_profile_hook = lambda h: setattr(m, "_hook", h)
    m.get_axon_ntff_profile_hook = lambda: m._hook
    sys.modules["antenv.axon_hooks"] = m
    antenv.axon_hooks = m
    try:
        from trn_agent_boot.trn_boot import _ntff_profile_via_ctypes

        hook = _ntff_profile_via_ctypes("/opt/axon/libaxon_pjrt.so")
        if hook is not None:
            m.set_axon_ntff_profile_hook(hook)
    except Exception:
        pass


_ensure_axon_hooks()

import concourse.bass as bass
import concourse.bacc as bacc
import concourse.mybir as mybir

N = 1_048_576
NCORES = 8
PC = N // NCORES          # 131072 rows per core
P = 128
F = PC // P               # 1024 slots per partition (partition-major rows)
COLS = 252
OC = 256                  # dense slot width in bf16 cols
OCW = OC // 2             # ... in u32 words
NUM_KNOTS = 256

PI = float(np.float32(np.pi))
H = float(np.float32(2.0 * np.pi / (NUM_KNOTS - 1)))
INVH = float(np.float32(1.0) / np.float32(H))
C1 = float(np.float32(PI - 0.5 * H))

AOT = mybir.AluOpType
AFT = mybir.ActivationFunctionType
F32 = mybir.dt.float32
BF16 = mybir.dt.bfloat16
U32 = mybir.dt.uint32

FC = 64                   # slots per chunk
NCHUNK = F // FC          # 16
WINW = 20                 # window width: 16-col block + 4 spill
NQ = 16                   # 16-col blocks per slot
BW = 8                    # u32 words per block

LN16 = float(np.float32(np.log(np.float32(1.0 / 6.0))))
LN23 = float(np.float32(np.log(np.float32(2.0 / 3.0))))


def build_nc():
    nc = bacc.Bacc("TRN2")
    w_d = nc.declare_dram_parameter("w", [P, F], F32, isOutput=False)
    m_d = nc.declare_dram_parameter("m", [P, F * NQ], U32, isOutput=False)
    out = nc.declare_dram_parameter("out", [PC, OC], BF16, isOutput=True)

    w_s = nc.alloc_sbuf_tensor("w_s", [P, F], F32)
    mb = [nc.alloc_sbuf_tensor(f"mb{i}", [P, FC * NQ], U32) for i in range(2)]
    iota20 = nc.alloc_sbuf_tensor("iota20", [P, FC * WINW], F32)
    bias_m2 = nc.alloc_sbuf_tensor("bias_m2", [P, 1], F32)
    bias_p2 = nc.alloc_sbuf_tensor("bias_p2", [P, 1], F32)
    bias_p1 = nc.alloc_sbuf_tensor("bias_p1", [P, 1], F32)
    bias_eps = nc.alloc_sbuf_tensor("bias_eps", [P, 1], F32)
    bias_ln6 = nc.alloc_sbuf_tensor("bias_ln6", [P, 1], F32)
    bias_ln23 = nc.alloc_sbuf_tensor("bias_ln23", [P, 1], F32)

    sb = [nc.alloc_sbuf_tensor(f"sb{i}", [P, FC * WINW], F32) for i in range(2)]
    zb = [nc.alloc_sbuf_tensor(f"zb{i}", [P, FC * WINW], F32) for i in range(2)]
    ub = [nc.alloc_sbuf_tensor(f"ub{i}", [P, FC * WINW], F32) for i in range(2)]
    z2b = [nc.alloc_sbuf_tensor(f"z2b{i}", [P, FC * WINW], F32) for i in range(2)]
    t2b = [nc.alloc_sbuf_tensor(f"t2b{i}", [P, FC * WINW], F32) for i in range(2)]
    winb = [nc.alloc_sbuf_tensor(f"winb{i}", [P, FC * WINW], BF16)
            for i in range(2)]
    spl = [nc.alloc_sbuf_tensor(f"spl{i}", [P, FC * 15 * 2], U32)
           for i in range(2)]
    dense = [nc.alloc_sbuf_tensor(f"dense{i}", [P, FC * OC], BF16)
             for i in range(2)]

    def wview(b):
        # u32 view of the 20 bf16 window cols: words 0..9
        return winb[b][:, :].bitcast(U32).rearrange("p (f j) -> p f j", j=10)

    def dqview(b):
        # u32 view grouped by 16-col block: [P, FC, 16, 8 words]
        return dense[b][:, :].bitcast(U32).rearrange(
            "p (f q j) -> p f q j", q=NQ, j=BW)

    def mview(b):
        return mb[b][:, :].rearrange("p (f q) -> p f q", q=NQ)

    with (
        nc.semaphore("insem") as insem,
        nc.semaphore("gsem") as gsem,
        nc.semaphore("msem") as msem,  # mask-chunk input DMA done
        nc.semaphore("sA") as sA,      # gpsimd s-grid ready
        nc.semaphore("sB") as sB,      # act y/z/t/z2/t2 ready
        nc.semaphore("sC") as sC,      # DVE win op done (act bufs free too)
        nc.semaphore("sDv") as sDv,    # DVE masked blocks + spill AND done
        nc.semaphore("csem") as csem,  # dense content final (incl. spill OR)
        nc.semaphore("dsem") as dsem,  # out-DMA completion
    ):
        with nc.Block() as block:

            @block.sync
            def _(s: bass.BassEngine):
                s.dma_start(out=w_s[:], in_=w_d[:, :]).then_inc(insem, 16)
                mv_d = m_d[:, :].rearrange("p (f q) -> p f q", q=NQ)
                for c in range(2):
                    s.dma_start(
                        out=mb[c][:],
                        in_=mv_d[:, c * FC:(c + 1) * FC, :],
                    ).then_inc(msem, 16)
                ov = out[:, :].rearrange("(p f) x -> p f x", p=P)
                for c in range(NCHUNK):
                    s.wait_ge(csem, c + 1)
                    s.dma_start(
                        out=ov[:, c * FC:(c + 1) * FC, :],
                        in_=dense[c % 2][:, :].rearrange(
                            "p (f x) -> p f x", x=OC),
                    ).then_inc(dsem, 16)
                    if c + 2 < NCHUNK:
                        # mb[c%2] free once csem(c) fired (all mask readers of
                        # chunk c are ordered before csem via sG -> OR)
                        s.dma_start(
                            out=mb[c % 2][:],
                            in_=mv_d[:, (c + 2) * FC:(c + 3) * FC, :],
                        ).then_inc(msem, 16)
                s.wait_ge(dsem, 16 * NCHUNK)

            @block.gpsimd
            def _(g: bass.BassEngine):
                g.memset(bias_m2[:], -2.0)
                g.memset(bias_p2[:], 2.0)
                g.memset(bias_p1[:], 1.0)
                g.memset(bias_eps[:], 1e-30)
                g.memset(bias_ln6[:], LN16)
                g.memset(bias_ln23[:], LN23)
                # j-grid 0..19 repeated per slot; f32 ints < 2^24 are exact
                g.iota(iota20[:], pattern=[[0, FC], [1, WINW]], base=0,
                       channel_multiplier=0,
                       allow_small_or_imprecise_dtypes=True).then_inc(gsem, 1)

            @block.scalar
            def _(a: bass.BassEngine):
                a.wait_ge(gsem, 1)  # bias tensors ready
                for c in range(NCHUNK):
                    a.wait_ge(sA, c + 1)
                    if c >= 2:
                        a.wait_ge(sC, c - 1)  # bufs[c%2] free
                    sbc, zbc, ubc = sb[c % 2], zb[c % 2], ub[c % 2]
                    z2c, t2c = z2b[c % 2], t2b[c % 2]
                    # y = |s-2| (in place), z = relu(2-y), t = relu(1-y)
                    # full cubes via exp(3 ln(.)+c): z2 = z^3/6, t2 = (2/3)t^3
                    # (the 1e-30 Ln bias keeps ln finite at z==0; exp of the
                    # scaled result underflows to exactly 0 there)
                    a.activation(out=sbc[:], in_=sbc[:], func=AFT.Abs,
                                 bias=bias_m2[:, :])
                    a.activation(out=zbc[:], in_=sbc[:], func=AFT.Relu,
                                 bias=bias_p2[:, :], scale=-1.0)
                    a.activation(out=ubc[:], in_=sbc[:], func=AFT.Relu,
                                 bias=bias_p1[:, :], scale=-1.0)
                    a.activation(out=z2c[:], in_=zbc[:], func=AFT.Ln,
                                 bias=bias_eps[:, :])
                    a.activation(out=z2c[:], in_=z2c[:], func=AFT.Exp,
                                 bias=bias_ln6[:, :], scale=3.0)
                    a.activation(out=t2c[:], in_=ubc[:], func=AFT.Ln,
                                 bias=bias_eps[:, :])
                    a.activation(out=t2c[:], in_=t2c[:], func=AFT.Exp,
                                 bias=bias_ln23[:, :],
                                 scale=3.0).then_inc(sB, 1)

            @block.vector
            def _(ve: bass.BassEngine):
                ve.wait_ge(gsem, 1)
                ve.wait_ge(insem, 16)

                def s_grid(c):
                    # s = w' - j over the 20-wide grid
                    ve.tensor_tensor(
                        out=sb[c % 2][:, :].rearrange(
                            "p (f j) -> p f j", j=WINW),
                        in0=w_s[:, c * FC:(c + 1) * FC].unsqueeze(2)
                        .broadcast_to([P, FC, WINW]),
                        in1=iota20[:, :].rearrange("p (f j) -> p f j", j=WINW),
                        op=AOT.subtract,
                    ).then_inc(sA, 1)

                s_grid(0)
                s_grid(1)
                for c in range(NCHUNK):
                    b = c % 2
                    ve.wait_ge(sB, c + 1)
                    if c + 2 < NCHUNK:
                        s_grid(c + 2)  # sb[b] free: act(c) finished with it
                    ve.tensor_tensor(out=winb[b][:], in0=z2b[b][:],
                                     in1=t2b[b][:],
                                     op=AOT.subtract).then_inc(sC, 1)
                    if c >= 2:
                        ve.wait_ge(dsem, 16 * (c - 1))  # dense[b] drained
                    ve.wait_ge(msem, 16 * (c + 1))      # mb[b] loaded
                    wv, dq, mv = wview(b), dqview(b), mview(b)
                    # all 16 block writes as ONE 4D-AP op: dense[f, q, 0:8] =
                    # win[f, 0:8] AND mbits[f, q] (win broadcast over q, mask
                    # broadcast over j)
                    ve.tensor_tensor(
                        out=dq[:, :, :, :],
                        in0=wv[:, :, 0:BW].unsqueeze(2).broadcast_to(
                            [P, FC, NQ, BW]),
                        in1=mv[:, :, :].unsqueeze(3).broadcast_to(
                            [P, FC, NQ, BW]),
                        op=AOT.bitwise_and,
                    )
                    # spill words: win u32 words 8:10 masked by the SOURCE
                    # block's mbits (q''=0..14), OR'd into block q''+1 (its
                    # first 2 words are 0 from the AND whenever the spill is
                    # nonzero, so OR is exact)
                    ve.tensor_tensor(
                        out=spl[b][:, :].rearrange(
                            "p (f q j) -> p f q j", q=15, j=2),
                        in0=wv[:, :, 8:10].unsqueeze(2).broadcast_to(
                            [P, FC, 15, 2]),
                        in1=mv[:, :, 0:15].unsqueeze(3).broadcast_to(
                            [P, FC, 15, 2]),
                        op=AOT.bitwise_and,
                    ).then_inc(sDv, 1)
                    # explicit edge before the overlapping OR: Bacc may
                    # reorder same-engine instructions whose strided-AP
                    # dependency it cannot see; the self-sem forces order.
                    ve.wait_ge(sDv, c + 1)
                    dq4 = dqview(b)
                    ve.tensor_tensor(
                        out=dq4[:, :, 1:16, 0:2],
                        in0=dq4[:, :, 1:16, 0:2],
                        in1=spl[b][:, :].rearrange(
                            "p (f q j) -> p f q j", q=15, j=2),
                        op=AOT.bitwise_or,
                    ).then_inc(csem, 1)

    nc.compile()
    return nc


_CACHED = {}


def make_in_maps(x: np.ndarray) -> list[dict]:
    xs = np.ascontiguousarray(np.asarray(x).reshape(N).astype(np.float32))
    u = (xs + np.float32(PI)) * np.float32(INVH)
    us = (xs + np.float32(C1)) * np.float32(INVH)
    ci = np.rint(us).astype(np.int64)          # == floor(u)
    cst = np.clip(ci - 3, 0, 248)
    du = u - cst.astype(np.float32)
    q = (cst >> 4).astype(np.int64)            # 16-col block index, 0..15
    of16 = (cst & 15).astype(np.float32)
    wv = du + of16                             # win[j] = C(wv - j), j in 0..19
    qbits = (q[:, None] == np.arange(NQ)[None, :])
    maps = []
    for c in range(NCORES):
        s = slice(c * PC, (c + 1) * PC)
        mbits = np.where(qbits[s], np.uint32(0xFFFFFFFF), np.uint32(0))
        maps.append({
            "w": np.ascontiguousarray(wv[s].reshape(P, F)),
            "m": np.ascontiguousarray(mbits.astype(np.uint32).reshape(
                P, F * NQ)),
        })
    return maps


def kernel(**inputs) -> np.ndarray:
    from concourse.bass_utils import run_bass_kernel_spmd

    x = np.asarray(inputs["x"], dtype=np.float32).reshape(N, 1)
    if "nc" not in _CACHED:
        _CACHED["nc"] = build_nc()
    nc = _CACHED["nc"]
    in_maps = make_in_maps(x)
    res = run_bass_kernel_spmd(nc, in_maps, list(range(NCORES)))
    return np.concatenate(
        [np.asarray(r["out"])[:, :COLS].astype(np.float32)
         for r in res.results],
        axis=0,
    )


if __name__ == "__main__":
    rng = np.random.default_rng(0)
    xs = rng.uniform(-np.pi, np.pi, size=(N, 1)).astype(np.float32)
    o = kernel(x=xs)
    print("out", o.shape, o.dtype, float(np.abs(o).max()))


# revision 19
# speedup vs baseline: 1.0529x; 1.0529x over previous
"""Trainium2 Bass kernel for uniform cubic B-spline basis (Cox-de Boor, degree 3).

Uniform knots => all 252 basis functions are shifts of one cardinal cubic C(s)
on [0,4). Row r is zero except columns cstart..cstart+3 (cstart = clip(i-3, 0,
248), i = floor(u), u = (x+pi)/h), holding C(u-cstart-m).

Strategy (dense + bitwise masks): the original kernel scatter-wrote
68-float windows with the Q7 `dma_scatter_add` ucode; its descriptor
generation (~7.5ns/token * 131072 tokens ~ 1ms) was the bottleneck. Instead,
each output row is materialized DENSELY in SBUF as a 256-wide bf16 slot
(cols 252..255 padding) and streamed out with plain contiguous HWDGE DMA
(67MB/core, DMA engines measured at ~25B/ns each).

Per row: q = cstart>>4 picks one of 16 16-col blocks; w' = (u-cstart) +
(cstart&15) is the eval point of a 20-wide window win[j] = C(w' - j) whose
4-wide support sits in [of16, of16+3] (spans at most blocks q, q+1). The
cubic: y=|s-2|, C = relu(2-y)^3/6 - (2/3)relu(1-y)^3; squares run on Act
(Square(z/sqrt6), Square(t*sqrt(2/3))), so DVE does 3 tensor_tensor ops.

Dense assembly uses BITWISE masking on uint32 lanes (2 bf16 cols per u32
element, halving element count vs bf16 masking): block q' of a slot =
win_u32[0:8] AND mbits[q'], where mbits = 0xFFFFFFFF iff q==q' (host-built,
streamed per chunk). The win[16:20] spill is OR'd into the first 2 u32 words
of block q'+1 (those words are 0 from the AND when the spill is nonzero, so
OR is exact; q'==15 never spills because right-edge clipping keeps of16<=8).
Bitwise ops are DVE-only (NCC_EBIR039: unsupported on the Pool engine), so
all masking runs on DVE; the overlapping spill OR sits behind a
self-semaphore so the strided-AP overlap is an explicit edge (Bacc's
dependency tracker does not model strided-AP overlaps and may reorder
instructions whose dependency it cannot see). The f32 window pipeline is
kept on DVE with abs/relu/Square on Act: offloading streaming tensor ops to
GPSIMD was tried and REGRESSED (343us -> 397us) because the Pool engine
shares its SBUF port with DVE; Pool must stay off streaming work. The 16
block writes are fused into one 4D-AP tensor_tensor to save instruction
overhead.

Row r maps partition-major (p = r // F, f = r % F) so each partition's chunk
is one contiguous 32KB DRAM run. Host prep is index arithmetic only
(u/cstart/w'/mask words); host post slices [:, :252] and casts bf16->f32
(values are computed in f32 and rounded once => rel err ~2e-3).
"""
import sys
import types

sys.path.insert(0, "/opt/trn_rl_repo")
sys.path.insert(0, "/root/.axon_site/_ro/trn_rl_repo")

import numpy as np


def _ensure_axon_hooks():
    if "antenv.axon_hooks" in sys.modules:
        return
    try:
        import antenv
    except ImportError:
        return
    m = types.ModuleType("antenv.axon_hooks")
    m._hook = None
    m.set_axon_ntff_profile_hook = lambda h: setattr(m, "_hook", h)
    m.get_axon_ntff_profile_hook = lambda: m._hook
    sys.modules["antenv.axon_hooks"] = m
    antenv.axon_hooks = m
    try:
        from trn_agent_boot.trn_boot import _ntff_profile_via_ctypes

        hook = _ntff_profile_via_ctypes("/opt/axon/libaxon_pjrt.so")
        if hook is not None:
            m.set_axon_ntff_profile_hook(hook)
    except Exception:
        pass


_ensure_axon_hooks()

import concourse.bass as bass
import concourse.bacc as bacc
import concourse.mybir as mybir

N = 1_048_576
NCORES = 8
PC = N // NCORES          # 131072 rows per core
P = 128
F = PC // P               # 1024 slots per partition (partition-major rows)
COLS = 252
OC = 256                  # dense slot width in bf16 cols
OCW = OC // 2             # ... in u32 words
NUM_KNOTS = 256

PI = float(np.float32(np.pi))
H = float(np.float32(2.0 * np.pi / (NUM_KNOTS - 1)))
INVH = float(np.float32(1.0) / np.float32(H))
C1 = float(np.float32(PI - 0.5 * H))

AOT = mybir.AluOpType
AFT = mybir.ActivationFunctionType
F32 = mybir.dt.float32
BF16 = mybir.dt.bfloat16
U32 = mybir.dt.uint32

FC = 64                   # slots per chunk
NCHUNK = F // FC          # 16
WINW = 20                 # window width: 16-col block + 4 spill
NQ = 16                   # 16-col blocks per slot
BW = 8                    # u32 words per block

ISQRT6 = float(np.float32(1.0) / np.float32(np.sqrt(np.float32(6.0))))
SQRT23 = float(np.float32(np.sqrt(np.float32(2.0 / 3.0))))


def build_nc():
    nc = bacc.Bacc("TRN2")
    w_d = nc.declare_dram_parameter("w", [P, F], F32, isOutput=False)
    m_d = nc.declare_dram_parameter("m", [P, F * NQ], U32, isOutput=False)
    out = nc.declare_dram_parameter("out", [PC, OC], BF16, isOutput=True)

    w_s = nc.alloc_sbuf_tensor("w_s", [P, F], F32)
    mb = [nc.alloc_sbuf_tensor(f"mb{i}", [P, FC * NQ], U32) for i in range(2)]
    iota20 = nc.alloc_sbuf_tensor("iota20", [P, FC * WINW], F32)
    bias_m2 = nc.alloc_sbuf_tensor("bias_m2", [P, 1], F32)
    bias_p2 = nc.alloc_sbuf_tensor("bias_p2", [P, 1], F32)
    bias_p1 = nc.alloc_sbuf_tensor("bias_p1", [P, 1], F32)
    bias_0 = nc.alloc_sbuf_tensor("bias_0", [P, 1], F32)

    sb = [nc.alloc_sbuf_tensor(f"sb{i}", [P, FC * WINW], F32) for i in range(2)]
    zb = [nc.alloc_sbuf_tensor(f"zb{i}", [P, FC * WINW], F32) for i in range(2)]
    ub = [nc.alloc_sbuf_tensor(f"ub{i}", [P, FC * WINW], F32) for i in range(2)]
    z2b = [nc.alloc_sbuf_tensor(f"z2b{i}", [P, FC * WINW], F32) for i in range(2)]
    t2b = [nc.alloc_sbuf_tensor(f"t2b{i}", [P, FC * WINW], F32) for i in range(2)]
    winb = [nc.alloc_sbuf_tensor(f"winb{i}", [P, FC * WINW], BF16)
            for i in range(2)]
    spl = [nc.alloc_sbuf_tensor(f"spl{i}", [P, FC * 15 * 2], U32)
           for i in range(2)]
    dense = [nc.alloc_sbuf_tensor(f"dense{i}", [P, FC * OC], BF16)
             for i in range(2)]

    def wview(b):
        # u32 view of the 20 bf16 window cols: words 0..9
        return winb[b][:, :].bitcast(U32).rearrange("p (f j) -> p f j", j=10)

    def dqview(b):
        # u32 view grouped by 16-col block: [P, FC, 16, 8 words]
        return dense[b][:, :].bitcast(U32).rearrange(
            "p (f q j) -> p f q j", q=NQ, j=BW)

    def mview(b):
        return mb[b][:, :].rearrange("p (f q) -> p f q", q=NQ)

    with (
        nc.semaphore("insem") as insem,
        nc.semaphore("gsem") as gsem,
        nc.semaphore("msem") as msem,  # mask-chunk input DMA done
        nc.semaphore("sA") as sA,      # gpsimd s-grid ready
        nc.semaphore("sB") as sB,      # act y/z/t/z2/t2 ready
        nc.semaphore("sC") as sC,      # DVE win op done (act bufs free too)
        nc.semaphore("sDv") as sDv,    # DVE masked blocks + spill AND done
        nc.semaphore("csem") as csem,  # dense content final (incl. spill OR)
        nc.semaphore("dsem") as dsem,  # out-DMA completion
    ):
        with nc.Block() as block:

            @block.sync
            def _(s: bass.BassEngine):
                s.dma_start(out=w_s[:], in_=w_d[:, :]).then_inc(insem, 16)
                mv_d = m_d[:, :].rearrange("p (f q) -> p f q", q=NQ)
                for c in range(2):
                    s.dma_start(
                        out=mb[c][:],
                        in_=mv_d[:, c * FC:(c + 1) * FC, :],
                    ).then_inc(msem, 16)
                ov = out[:, :].rearrange("(p f) x -> p f x", p=P)
                for c in range(NCHUNK):
                    s.wait_ge(csem, c + 1)
                    s.dma_start(
                        out=ov[:, c * FC:(c + 1) * FC, :],
                        in_=dense[c % 2][:, :].rearrange(
                            "p (f x) -> p f x", x=OC),
                    ).then_inc(dsem, 16)
                    if c + 2 < NCHUNK:
                        # mb[c%2] free once csem(c) fired (all mask readers of
                        # chunk c are ordered before csem via sG -> OR)
                        s.dma_start(
                            out=mb[c % 2][:],
                            in_=mv_d[:, (c + 2) * FC:(c + 3) * FC, :],
                        ).then_inc(msem, 16)
                s.wait_ge(dsem, 16 * NCHUNK)

            @block.gpsimd
            def _(g: bass.BassEngine):
                g.memset(bias_m2[:], -2.0)
                g.memset(bias_p2[:], 2.0)
                g.memset(bias_p1[:], 1.0)
                g.memset(bias_0[:], 0.0)
                # j-grid 0..19 repeated per slot; f32 ints < 2^24 are exact
                g.iota(iota20[:], pattern=[[0, FC], [1, WINW]], base=0,
                       channel_multiplier=0,
                       allow_small_or_imprecise_dtypes=True).then_inc(gsem, 1)

            @block.scalar
            def _(a: bass.BassEngine):
                a.wait_ge(gsem, 1)  # bias tensors ready
                for c in range(NCHUNK):
                    a.wait_ge(sA, c + 1)
                    if c >= 2:
                        a.wait_ge(sC, c - 1)  # bufs[c%2] free
                    sbc, zbc, ubc = sb[c % 2], zb[c % 2], ub[c % 2]
                    z2c, t2c = z2b[c % 2], t2b[c % 2]
                    # y = |s-2| (in place), z = relu(2-y), t = relu(1-y)
                    # z2 = z^2/6 (Square of z/sqrt6), t2 = (2/3) t^2
                    # (Abs/Relu/Square share one act table set; Ln/Exp cubes
                    # were tried and REGRESSED: 4 ACT_TABLE_LOADs per chunk,
                    # +100us of table thrash)
                    a.activation(out=sbc[:], in_=sbc[:], func=AFT.Abs,
                                 bias=bias_m2[:, :])
                    a.activation(out=zbc[:], in_=sbc[:], func=AFT.Relu,
                                 bias=bias_p2[:, :], scale=-1.0)
                    a.activation(out=ubc[:], in_=sbc[:], func=AFT.Relu,
                                 bias=bias_p1[:, :], scale=-1.0)
                    a.activation(out=z2c[:], in_=zbc[:], func=AFT.Square,
                                 bias=bias_0[:, :], scale=ISQRT6)
                    a.activation(out=t2c[:], in_=ubc[:], func=AFT.Square,
                                 bias=bias_0[:, :],
                                 scale=SQRT23).then_inc(sB, 1)

            @block.vector
            def _(ve: bass.BassEngine):
                ve.wait_ge(gsem, 1)
                ve.wait_ge(insem, 16)

                def s_grid(c):
                    # s = w' - j over the 20-wide grid
                    ve.tensor_tensor(
                        out=sb[c % 2][:, :].rearrange(
                            "p (f j) -> p f j", j=WINW),
                        in0=w_s[:, c * FC:(c + 1) * FC].unsqueeze(2)
                        .broadcast_to([P, FC, WINW]),
                        in1=iota20[:, :].rearrange("p (f j) -> p f j", j=WINW),
                        op=AOT.subtract,
                    ).then_inc(sA, 1)

                s_grid(0)
                s_grid(1)
                for c in range(NCHUNK):
                    b = c % 2
                    ve.wait_ge(sB, c + 1)
                    if c + 2 < NCHUNK:
                        s_grid(c + 2)  # sb[b] free: act(c) finished with it
                    zbc, ubc = zb[b], ub[b]
                    z2c, t2c = z2b[b], t2b[b]
                    # zc = z^3/6 (in place on z2), tc = (2/3) t^3 (in place)
                    ve.tensor_tensor(out=z2c[:], in0=z2c[:], in1=zbc[:],
                                     op=AOT.mult)
                    ve.tensor_tensor(out=t2c[:], in0=t2c[:], in1=ubc[:],
                                     op=AOT.mult)
                    ve.tensor_tensor(out=winb[b][:], in0=z2c[:], in1=t2c[:],
                                     op=AOT.subtract).then_inc(sC, 1)
                    if c >= 2:
                        ve.wait_ge(dsem, 16 * (c - 1))  # dense[b] drained
                    ve.wait_ge(msem, 16 * (c + 1))      # mb[b] loaded
                    wv, dq, mv = wview(b), dqview(b), mview(b)
                    # all 16 block writes as ONE 4D-AP op: dense[f, q, 0:8] =
                    # win[f, 0:8] AND mbits[f, q] (win broadcast over q, mask
                    # broadcast over j)
                    ve.tensor_tensor(
                        out=dq[:, :, :, :],
                        in0=wv[:, :, 0:BW].unsqueeze(2).broadcast_to(
                            [P, FC, NQ, BW]),
                        in1=mv[:, :, :].unsqueeze(3).broadcast_to(
                            [P, FC, NQ, BW]),
                        op=AOT.bitwise_and,
                    )
                    # spill words: win u32 words 8:10 masked by the SOURCE
                    # block's mbits (q''=0..14), OR'd into block q''+1 (its
                    # first 2 words are 0 from the AND whenever the spill is
                    # nonzero, so OR is exact)
                    ve.tensor_tensor(
                        out=spl[b][:, :].rearrange(
                            "p (f q j) -> p f q j", q=15, j=2),
                        in0=wv[:, :, 8:10].unsqueeze(2).broadcast_to(
                            [P, FC, 15, 2]),
                        in1=mv[:, :, 0:15].unsqueeze(3).broadcast_to(
                            [P, FC, 15, 2]),
                        op=AOT.bitwise_and,
                    ).then_inc(sDv, 1)
                    # explicit edge before the overlapping OR: Bacc may
                    # reorder same-engine instructions whose strided-AP
                    # dependency it cannot see; the self-sem forces order.
                    ve.wait_ge(sDv, c + 1)
                    dq4 = dqview(b)
                    ve.tensor_tensor(
                        out=dq4[:, :, 1:16, 0:2],
                        in0=dq4[:, :, 1:16, 0:2],
                        in1=spl[b][:, :].rearrange(
                            "p (f q j) -> p f q j", q=15, j=2),
                        op=AOT.bitwise_or,
                    ).then_inc(csem, 1)

    nc.compile()
    return nc


_CACHED = {}


def make_in_maps(x: np.ndarray) -> list[dict]:
    xs = np.ascontiguousarray(np.asarray(x).reshape(N).astype(np.float32))
    u = (xs + np.float32(PI)) * np.float32(INVH)
    us = (xs + np.float32(C1)) * np.float32(INVH)
    ci = np.rint(us).astype(np.int64)          # == floor(u)
    cst = np.clip(ci - 3, 0, 248)
    du = u - cst.astype(np.float32)
    q = (cst >> 4).astype(np.int64)            # 16-col block index, 0..15
    of16 = (cst & 15).astype(np.float32)
    wv = du + of16                             # win[j] = C(wv - j), j in 0..19
    qbits = (q[:, None] == np.arange(NQ)[None, :])
    maps = []
    for c in range(NCORES):
        s = slice(c * PC, (c + 1) * PC)
        mbits = np.where(qbits[s], np.uint32(0xFFFFFFFF), np.uint32(0))
        maps.append({
            "w": np.ascontiguousarray(wv[s].reshape(P, F)),
            "m": np.ascontiguousarray(mbits.astype(np.uint32).reshape(
                P, F * NQ)),
        })
    return maps


def kernel(**inputs) -> np.ndarray:
    from concourse.bass_utils import run_bass_kernel_spmd

    x = np.asarray(inputs["x"], dtype=np.float32).reshape(N, 1)
    if "nc" not in _CACHED:
        _CACHED["nc"] = build_nc()
    nc = _CACHED["nc"]
    in_maps = make_in_maps(x)
    res = run_bass_kernel_spmd(nc, in_maps, list(range(NCORES)))
    return np.concatenate(
        [np.asarray(r["out"])[:, :COLS].astype(np.float32)
         for r in res.results],
        axis=0,
    )


if __name__ == "__main__":
    rng = np.random.default_rng(0)
    xs = rng.uniform(-np.pi, np.pi, size=(N, 1)).astype(np.float32)
    o = kernel(x=xs)
    print("out", o.shape, o.dtype, float(np.abs(o).max()))


# revision 20
# speedup vs baseline: 1.1644x; 1.1058x over previous
"""Trainium2 Bass kernel for uniform cubic B-spline basis (Cox-de Boor, degree 3).

Uniform knots => all 252 basis functions are shifts of one cardinal cubic C(s)
on [0,4). Row r is zero except columns cstart..cstart+3 (cstart = clip(i-3, 0,
248), i = floor(u), u = (x+pi)/h), holding C(u-cstart-m).

Strategy (dense + bitwise masks): the original kernel scatter-wrote
68-float windows with the Q7 `dma_scatter_add` ucode; its descriptor
generation (~7.5ns/token * 131072 tokens ~ 1ms) was the bottleneck. Instead,
each output row is materialized DENSELY in SBUF as a 256-wide bf16 slot
(cols 252..255 padding) and streamed out with plain contiguous HWDGE DMA
(67MB/core, DMA engines measured at ~25B/ns each).

Per row: q = cstart>>4 picks one of 16 16-col blocks; w' = (u-cstart) +
(cstart&15) is the eval point of a 20-wide window win[j] = C(w' - j) whose
4-wide support sits in [of16, of16+3] (spans at most blocks q, q+1). The
cubic: y=|s-2|, C = relu(2-y)^3/6 - (2/3)relu(1-y)^3; squares run on Act
(Square(z/sqrt6), Square(t*sqrt(2/3))), so DVE does 3 tensor_tensor ops.
z/t/z2/t2 are bf16 so those three DVE ops are all-2B unit-stride and
eligible for the DVE 2x packed mode (the s-grid stays f32 for range
precision; total error stays ~5e-3 fro, gate is 2e-2).

Dense assembly uses BITWISE masking on uint32 lanes (2 bf16 cols per u32
element, halving element count vs bf16 masking): block q' of a slot =
win_u32[0:8] AND mbits[q'], where mbits = 0xFFFFFFFF iff q==q' (host-built,
streamed per chunk). The win[16:20] spill is OR'd into the first 2 u32 words
of block q'+1 (those words are 0 from the AND when the spill is nonzero, so
OR is exact; q'==15 never spills because right-edge clipping keeps of16<=8).
Bitwise ops are DVE-only (NCC_EBIR039: unsupported on the Pool engine), so
all masking runs on DVE; the overlapping spill OR sits behind a
self-semaphore so the strided-AP overlap is an explicit edge (Bacc's
dependency tracker does not model strided-AP overlaps and may reorder
instructions whose dependency it cannot see). The f32 window pipeline is
kept on DVE with abs/relu/Square on Act: offloading streaming tensor ops to
GPSIMD was tried and REGRESSED (343us -> 397us) because the Pool engine
shares its SBUF port with DVE; Pool must stay off streaming work. The 16
block writes are fused into one 4D-AP tensor_tensor to save instruction
overhead.

Row r maps partition-major (p = r // F, f = r % F) so each partition's chunk
is one contiguous 32KB DRAM run. Host prep is index arithmetic only
(u/cstart/w'/mask words); host post slices [:, :252] and casts bf16->f32
(values are computed in f32 and rounded once => rel err ~2e-3).
"""
import sys
import types

sys.path.insert(0, "/opt/trn_rl_repo")
sys.path.insert(0, "/root/.axon_site/_ro/trn_rl_repo")

import numpy as np


def _ensure_axon_hooks():
    if "antenv.axon_hooks" in sys.modules:
        return
    try:
        import antenv
    except ImportError:
        return
    m = types.ModuleType("antenv.axon_hooks")
    m._hook = None
    m.set_axon_ntff_profile_hook = lambda h: setattr(m, "_hook", h)
    m.get_axon_ntff_profile_hook = lambda: m._hook
    sys.modules["antenv.axon_hooks"] = m
    antenv.axon_hooks = m
    try:
        from trn_agent_boot.trn_boot import _ntff_profile_via_ctypes

        hook = _ntff_profile_via_ctypes("/opt/axon/libaxon_pjrt.so")
        if hook is not None:
            m.set_axon_ntff_profile_hook(hook)
    except Exception:
        pass


_ensure_axon_hooks()

import concourse.bass as bass
import concourse.bacc as bacc
import concourse.mybir as mybir

N = 1_048_576
NCORES = 8
PC = N // NCORES          # 131072 rows per core
P = 128
F = PC // P               # 1024 slots per partition (partition-major rows)
COLS = 252
OC = 256                  # dense slot width in bf16 cols
OCW = OC // 2             # ... in u32 words
NUM_KNOTS = 256

PI = float(np.float32(np.pi))
H = float(np.float32(2.0 * np.pi / (NUM_KNOTS - 1)))
INVH = float(np.float32(1.0) / np.float32(H))
C1 = float(np.float32(PI - 0.5 * H))

AOT = mybir.AluOpType
AFT = mybir.ActivationFunctionType
F32 = mybir.dt.float32
BF16 = mybir.dt.bfloat16
U32 = mybir.dt.uint32

FC = 64                   # slots per chunk
NCHUNK = F // FC          # 16
WINW = 20                 # window width: 16-col block + 4 spill
NQ = 16                   # 16-col blocks per slot
BW = 8                    # u32 words per block

ISQRT6 = float(np.float32(1.0) / np.float32(np.sqrt(np.float32(6.0))))
SQRT23 = float(np.float32(np.sqrt(np.float32(2.0 / 3.0))))


def build_nc():
    nc = bacc.Bacc("TRN2")
    w_d = nc.declare_dram_parameter("w", [P, F], F32, isOutput=False)
    m_d = nc.declare_dram_parameter("m", [P, F * NQ], U32, isOutput=False)
    out = nc.declare_dram_parameter("out", [PC, OC], BF16, isOutput=True)

    w_s = nc.alloc_sbuf_tensor("w_s", [P, F], F32)
    mb = [nc.alloc_sbuf_tensor(f"mb{i}", [P, FC * NQ], U32) for i in range(2)]
    iota20 = nc.alloc_sbuf_tensor("iota20", [P, FC * WINW], F32)
    bias_m2 = nc.alloc_sbuf_tensor("bias_m2", [P, 1], F32)
    bias_p2 = nc.alloc_sbuf_tensor("bias_p2", [P, 1], F32)
    bias_p1 = nc.alloc_sbuf_tensor("bias_p1", [P, 1], F32)
    bias_0 = nc.alloc_sbuf_tensor("bias_0", [P, 1], F32)

    sb = [nc.alloc_sbuf_tensor(f"sb{i}", [P, FC * WINW], F32) for i in range(2)]
    zb = [nc.alloc_sbuf_tensor(f"zb{i}", [P, FC * WINW], BF16)
          for i in range(2)]
    ub = [nc.alloc_sbuf_tensor(f"ub{i}", [P, FC * WINW], BF16)
          for i in range(2)]
    z2b = [nc.alloc_sbuf_tensor(f"z2b{i}", [P, FC * WINW], BF16)
           for i in range(2)]
    t2b = [nc.alloc_sbuf_tensor(f"t2b{i}", [P, FC * WINW], BF16)
           for i in range(2)]
    winb = [nc.alloc_sbuf_tensor(f"winb{i}", [P, FC * WINW], BF16)
            for i in range(2)]
    spl = [nc.alloc_sbuf_tensor(f"spl{i}", [P, FC * 15 * 2], U32)
           for i in range(2)]
    dense = [nc.alloc_sbuf_tensor(f"dense{i}", [P, FC * OC], BF16)
             for i in range(2)]

    def wview(b):
        # u32 view of the 20 bf16 window cols: words 0..9
        return winb[b][:, :].bitcast(U32).rearrange("p (f j) -> p f j", j=10)

    def dqview(b):
        # u32 view grouped by 16-col block: [P, FC, 16, 8 words]
        return dense[b][:, :].bitcast(U32).rearrange(
            "p (f q j) -> p f q j", q=NQ, j=BW)

    def mview(b):
        return mb[b][:, :].rearrange("p (f q) -> p f q", q=NQ)

    with (
        nc.semaphore("insem") as insem,
        nc.semaphore("gsem") as gsem,
        nc.semaphore("msem") as msem,  # mask-chunk input DMA done
        nc.semaphore("sA") as sA,      # gpsimd s-grid ready
        nc.semaphore("sB") as sB,      # act y/z/t/z2/t2 ready
        nc.semaphore("sC") as sC,      # DVE win op done (act bufs free too)
        nc.semaphore("sDv") as sDv,    # DVE masked blocks + spill AND done
        nc.semaphore("csem") as csem,  # dense content final (incl. spill OR)
        nc.semaphore("dsem") as dsem,  # out-DMA completion
    ):
        with nc.Block() as block:

            @block.sync
            def _(s: bass.BassEngine):
                s.dma_start(out=w_s[:], in_=w_d[:, :]).then_inc(insem, 16)
                mv_d = m_d[:, :].rearrange("p (f q) -> p f q", q=NQ)
                for c in range(2):
                    s.dma_start(
                        out=mb[c][:],
                        in_=mv_d[:, c * FC:(c + 1) * FC, :],
                    ).then_inc(msem, 16)
                ov = out[:, :].rearrange("(p f) x -> p f x", p=P)
                for c in range(NCHUNK):
                    s.wait_ge(csem, c + 1)
                    s.dma_start(
                        out=ov[:, c * FC:(c + 1) * FC, :],
                        in_=dense[c % 2][:, :].rearrange(
                            "p (f x) -> p f x", x=OC),
                    ).then_inc(dsem, 16)
                    if c + 2 < NCHUNK:
                        # mb[c%2] free once csem(c) fired (all mask readers of
                        # chunk c are ordered before csem via sG -> OR)
                        s.dma_start(
                            out=mb[c % 2][:],
                            in_=mv_d[:, (c + 2) * FC:(c + 3) * FC, :],
                        ).then_inc(msem, 16)
                s.wait_ge(dsem, 16 * NCHUNK)

            @block.gpsimd
            def _(g: bass.BassEngine):
                g.memset(bias_m2[:], -2.0)
                g.memset(bias_p2[:], 2.0)
                g.memset(bias_p1[:], 1.0)
                g.memset(bias_0[:], 0.0)
                # j-grid 0..19 repeated per slot; f32 ints < 2^24 are exact
                g.iota(iota20[:], pattern=[[0, FC], [1, WINW]], base=0,
                       channel_multiplier=0,
                       allow_small_or_imprecise_dtypes=True).then_inc(gsem, 1)

            @block.scalar
            def _(a: bass.BassEngine):
                a.wait_ge(gsem, 1)  # bias tensors ready
                for c in range(NCHUNK):
                    a.wait_ge(sA, c + 1)
                    if c >= 2:
                        a.wait_ge(sC, c - 1)  # bufs[c%2] free
                    sbc, zbc, ubc = sb[c % 2], zb[c % 2], ub[c % 2]
                    z2c, t2c = z2b[c % 2], t2b[c % 2]
                    # y = |s-2| (in place), z = relu(2-y), t = relu(1-y)
                    # z2 = z^2/6 (Square of z/sqrt6), t2 = (2/3) t^2
                    # (Abs/Relu/Square share one act table set; Ln/Exp cubes
                    # were tried and REGRESSED: 4 ACT_TABLE_LOADs per chunk,
                    # +100us of table thrash)
                    a.activation(out=sbc[:], in_=sbc[:], func=AFT.Abs,
                                 bias=bias_m2[:, :])
                    a.activation(out=zbc[:], in_=sbc[:], func=AFT.Relu,
                                 bias=bias_p2[:, :], scale=-1.0)
                    a.activation(out=ubc[:], in_=sbc[:], func=AFT.Relu,
                                 bias=bias_p1[:, :], scale=-1.0)
                    a.activation(out=z2c[:], in_=zbc[:], func=AFT.Square,
                                 bias=bias_0[:, :], scale=ISQRT6)
                    a.activation(out=t2c[:], in_=ubc[:], func=AFT.Square,
                                 bias=bias_0[:, :],
                                 scale=SQRT23).then_inc(sB, 1)

            @block.vector
            def _(ve: bass.BassEngine):
                ve.wait_ge(gsem, 1)
                ve.wait_ge(insem, 16)

                def s_grid(c):
                    # s = w' - j over the 20-wide grid
                    ve.tensor_tensor(
                        out=sb[c % 2][:, :].rearrange(
                            "p (f j) -> p f j", j=WINW),
                        in0=w_s[:, c * FC:(c + 1) * FC].unsqueeze(2)
                        .broadcast_to([P, FC, WINW]),
                        in1=iota20[:, :].rearrange("p (f j) -> p f j", j=WINW),
                        op=AOT.subtract,
                    ).then_inc(sA, 1)

                s_grid(0)
                s_grid(1)
                for c in range(NCHUNK):
                    b = c % 2
                    ve.wait_ge(sB, c + 1)
                    if c + 2 < NCHUNK:
                        s_grid(c + 2)  # sb[b] free: act(c) finished with it
                    zbc, ubc = zb[b], ub[b]
                    z2c, t2c = z2b[b], t2b[b]
                    # zc = z^3/6 (in place on z2), tc = (2/3) t^3 (in place)
                    ve.tensor_tensor(out=z2c[:], in0=z2c[:], in1=zbc[:],
                                     op=AOT.mult)
                    ve.tensor_tensor(out=t2c[:], in0=t2c[:], in1=ubc[:],
                                     op=AOT.mult)
                    ve.tensor_tensor(out=winb[b][:], in0=z2c[:], in1=t2c[:],
                                     op=AOT.subtract).then_inc(sC, 1)
                    if c >= 2:
                        ve.wait_ge(dsem, 16 * (c - 1))  # dense[b] drained
                    ve.wait_ge(msem, 16 * (c + 1))      # mb[b] loaded
                    wv, dq, mv = wview(b), dqview(b), mview(b)
                    # all 16 block writes as ONE 4D-AP op: dense[f, q, 0:8] =
                    # win[f, 0:8] AND mbits[f, q] (win broadcast over q, mask
                    # broadcast over j)
                    ve.tensor_tensor(
                        out=dq[:, :, :, :],
                        in0=wv[:, :, 0:BW].unsqueeze(2).broadcast_to(
                            [P, FC, NQ, BW]),
                        in1=mv[:, :, :].unsqueeze(3).broadcast_to(
                            [P, FC, NQ, BW]),
                        op=AOT.bitwise_and,
                    )
                    # spill words: win u32 words 8:10 masked by the SOURCE
                    # block's mbits (q''=0..14), OR'd into block q''+1 (its
                    # first 2 words are 0 from the AND whenever the spill is
                    # nonzero, so OR is exact)
                    ve.tensor_tensor(
                        out=spl[b][:, :].rearrange(
                            "p (f q j) -> p f q j", q=15, j=2),
                        in0=wv[:, :, 8:10].unsqueeze(2).broadcast_to(
                            [P, FC, 15, 2]),
                        in1=mv[:, :, 0:15].unsqueeze(3).broadcast_to(
                            [P, FC, 15, 2]),
                        op=AOT.bitwise_and,
                    ).then_inc(sDv, 1)
                    # explicit edge before the overlapping OR: Bacc may
                    # reorder same-engine instructions whose strided-AP
                    # dependency it cannot see; the self-sem forces order.
                    ve.wait_ge(sDv, c + 1)
                    dq4 = dqview(b)
                    ve.tensor_tensor(
                        out=dq4[:, :, 1:16, 0:2],
                        in0=dq4[:, :, 1:16, 0:2],
                        in1=spl[b][:, :].rearrange(
                            "p (f q j) -> p f q j", q=15, j=2),
                        op=AOT.bitwise_or,
                    ).then_inc(csem, 1)

    nc.compile()
    return nc


_CACHED = {}


def make_in_maps(x: np.ndarray) -> list[dict]:
    xs = np.ascontiguousarray(np.asarray(x).reshape(N).astype(np.float32))
    u = (xs + np.float32(PI)) * np.float32(INVH)
    us = (xs + np.float32(C1)) * np.float32(INVH)
    ci = np.rint(us).astype(np.int64)          # == floor(u)
    cst = np.clip(ci - 3, 0, 248)
    du = u - cst.astype(np.float32)
    q = (cst >> 4).astype(np.int64)            # 16-col block index, 0..15
    of16 = (cst & 15).astype(np.float32)
    wv = du + of16                             # win[j] = C(wv - j), j in 0..19
    qbits = (q[:, None] == np.arange(NQ)[None, :])
    maps = []
    for c in range(NCORES):
        s = slice(c * PC, (c + 1) * PC)
        mbits = np.where(qbits[s], np.uint32(0xFFFFFFFF), np.uint32(0))
        maps.append({
            "w": np.ascontiguousarray(wv[s].reshape(P, F)),
            "m": np.ascontiguousarray(mbits.astype(np.uint32).reshape(
                P, F * NQ)),
        })
    return maps


def kernel(**inputs) -> np.ndarray:
    from concourse.bass_utils import run_bass_kernel_spmd

    x = np.asarray(inputs["x"], dtype=np.float32).reshape(N, 1)
    if "nc" not in _CACHED:
        _CACHED["nc"] = build_nc()
    nc = _CACHED["nc"]
    in_maps = make_in_maps(x)
    res = run_bass_kernel_spmd(nc, in_maps, list(range(NCORES)))
    return np.concatenate(
        [np.asarray(r["out"])[:, :COLS].astype(np.float32)
         for r in res.results],
        axis=0,
    )


if __name__ == "__main__":
    rng = np.random.default_rng(0)
    xs = rng.uniform(-np.pi, np.pi, size=(N, 1)).astype(np.float32)
    o = kernel(x=xs)
    print("out", o.shape, o.dtype, float(np.abs(o).max()))
